# revision 105
# baseline (speedup 1.0000x reference)
"""MultiHeadAttention Trainium2 kernel (8-core SPMD, head/tensor parallel).

Problem (hardcoded shapes): stream (2048, 2, 1024) f32, mask (1, 2048, 2048),
w_qkv (1024, 3072), b_qkv (3072,), w_out (1024, 1024), b_out (1024,).
N=2048, B=2, HEADS=16, D_KQ=D_V=64, D_HEAD=192.

Sharding (per the b*heads head-parallel hint): core d handles batch b=d//4 and
the 4 heads [4*(d%4), 4*(d%4)+4): w_qkv columns and w_out rows are split per
head group, logits/weights are fully local per core, and the post-projection
all-reduce (sum over the 4 cores of each batch plus the two per-pair partial
outputs, + b_out) is done on the host during unsharding.

Schedule (engine-balanced, derived from the TimelineSim cost model):
  - ACT only runs the 128 softmax exps ([128,1024] each, ~133us total); all
    bias-adds and PSUM->SBUF copies live on DVE/Pool so ACT is never stalled.
  - Projection inputs (xT/w_qkv/w_v) are bf16: halves the startup DMA and
    keeps PE continuously busy (p-state ramp).
  - QKV projection for the first head pair runs kt-major across four
    [128,1024] PSUM tiles so each arriving xT k-slice is consumed at once.
  - Attention runs as 8 windows of (pair, chunk, head): single PV accumulator
    per window leaves PSUM tag F free for interleaved filler matmuls (second
    pair's q/k projection, per-pair output projection) that soak up the
    PE idle gap while ACT streams exps.
  - exp(mask^T) is SBUF-resident (loaded once, bf16, applied as a DVE 2x/4x
    multiply); softmax denominators come free from ones-columns in the PV
    lhsT; reciprocal on DVE with a DMA partition-move.
  - Output projection is per head-pair (no cross-pair PSUM accumulation) so
    each pair's contribution streams out as soon as that pair's values are
    normalized; the host sums the two partial outputs per core.
"""

import numpy as np
import ml_dtypes

import concourse.tile as tile
from concourse import bacc, mybir
from concourse.bass_utils import run_bass_kernel_spmd

BF16 = ml_dtypes.bfloat16
dt = mybir.dt
AF = mybir.ActivationFunctionType

# Shapes (hardcoded per the problem spec)
N = 2048          # sequence length
B = 2             # batch
DSTR = 1024       # d_stream
HEADS = 16        # total heads
NH = 4            # heads per core
DKQ = 64
DV = 64
DHEAD = 2 * DKQ + DV
P = 128
KT = DSTR // P    # 8 contraction k-tiles for projections
MT = N // P       # 16 m-tiles
CH = 1024         # attention n-chunk width
NCH = N // CH     # 2 chunks
NB = 512          # matmul moving free dim
N_CORES = 8

f32, f32r, bf16 = dt.float32, dt.float32r, dt.bfloat16

_BUILT = {}


def _build_nc():
    """Build + compile the single-core SPMD Bass program (same on all cores)."""
    nc = bacc.Bacc("TRN2", target_bir_lowering=False, debug=False)

    xT = nc.dram_tensor("xT", [DSTR, N], bf16, kind="ExternalInput").ap()
    wqk = nc.dram_tensor("wqk", [DSTR, 4 * P], bf16, kind="ExternalInput").ap()
    wv = nc.dram_tensor("wv", [DSTR, NH * DV], bf16, kind="ExternalInput").ap()
    bqk = nc.dram_tensor("bqk", [P, 4], f32, kind="ExternalInput").ap()
    bv = nc.dram_tensor("bv", [1, NH * DV], bf16, kind="ExternalInput").ap()
    ones = nc.dram_tensor("ones", [1, P], bf16, kind="ExternalInput").ap()
    emT = nc.dram_tensor("emT", [N, N], bf16, kind="ExternalInput").ap()
    wout = nc.dram_tensor("wout", [NH * DV, DSTR], bf16, kind="ExternalInput").ap()
    ident = nc.dram_tensor("ident", [P, 64], f32, kind="ExternalInput").ap()
    outp = [
        nc.dram_tensor(f"out{i}", [N, DSTR], bf16, kind="ExternalOutput").ap()
        for i in range(2)
    ]

    with tile.TileContext(nc) as tc:
        with (
            tc.tile_pool(name="consts", bufs=1) as consts,
            tc.tile_pool(name="xw", bufs=1) as xw_p,
            tc.tile_pool(name="qkT", bufs=1) as qkT_p,
            tc.tile_pool(name="v1", bufs=1) as v1_p,
            tc.tile_pool(name="valT", bufs=1) as valT_p,
            tc.tile_pool(name="em", bufs=1) as em_p,
            tc.tile_pool(name="wT", bufs=7) as wT_p,
            tc.tile_pool(name="z", bufs=2) as z_p,
            tc.tile_pool(name="stage", bufs=8) as stage_p,
            tc.tile_pool(name="ps", bufs=1, space="PSUM") as ps_p,
        ):
            # ---------- persistent SBUF ----------
            xT_sb = xw_p.tile([P, KT, N], bf16)
            wqk_sb = xw_p.tile([P, KT, 4 * P], bf16)
            wv_sb = xw_p.tile([P, KT, NH * DV], bf16)
            # the kt-major projection gating window 0 needs only wqk+xT:
            # wv (needed first by the proj_v seeds, ~1.5us later) loads after
            for kt in range(KT):
                nc.sync.dma_start(out=wqk_sb[:, kt, :], in_=wqk[kt * P:(kt + 1) * P, :])
                nc.sync.dma_start(out=xT_sb[:, kt, :], in_=xT[kt * P:(kt + 1) * P, :])
            bqk_sb = consts.tile([P, 4], f32)
            nc.sync.dma_start(out=bqk_sb, in_=bqk)
            # pre-warm the ACT exp table off the critical path
            warm = consts.tile([P, 1], f32)
            nc.scalar.activation(out=warm, in_=bqk_sb[:, 0:1], func=AF.Exp)
            for kt in range(KT):
                nc.sync.dma_start(out=wv_sb[:, kt, :], in_=wv[kt * P:(kt + 1) * P, :])
            ones1 = consts.tile([1, P], bf16)
            nc.sync.dma_start(out=ones1, in_=ones)
            bv_sb = consts.tile([1, NH * DV], bf16)
            nc.sync.dma_start(out=bv_sb, in_=bv)

            # exp(mask^T), SBUF-resident for the whole kernel; chunk-0 quads
            # load BEFORE wout/ident (not needed until ~65us) so window 0's
            # first mask-multiplies aren't gated on mask arrival
            em_sb = em_p.tile([P, NCH, MT, CH], bf16)

            def em_load(c, groups=((0, 4), (4, 8), (8, 12), (12, 16))):
                for m0, m1 in groups:
                    nc.sync.dma_start(
                        out=em_sb[:, c, m0:m1, :],
                        in_=emT[m0 * P:m1 * P,
                                c * CH:(c + 1) * CH].rearrange(
                                    "(t p) n -> p t n", p=P),
                    )

            # first two mask tiles in a small transfer: window 0's first
            # multiply wants them ~1.5us before a full quad could land
            em_load(0, groups=((0, 2), (2, 4), (4, 8), (8, 12), (12, 16)))
            wout_sb = consts.tile([P, 2, DSTR], bf16)
            nc.sync.dma_start(
                out=wout_sb, in_=wout.rearrange("(i p) d -> p i d", p=P))
            # 64x64 identity in both partition halves: PE-based partition
            # move of the final window's reciprocal block
            ident_sb = consts.tile([P, 64], f32)
            nc.sync.dma_start(out=ident_sb, in_=ident)
            em_load(1)

            # qkT f-tiles: 0 = q pair0, 1 = q pair1, 2 = k pair0, 3 = k pair1
            # (within a tile: partitions 0:64 = even head's d, 64:128 = odd's)
            qkT = qkT_p.tile([P, 4, N], bf16)
            # v1 lhsT slots per (mt, head): even-in-pair = [v | ones],
            # odd-in-pair = [ones | v] -> PV output carries values rows and
            # 64x-replicated Z rows in complementary partition halves.
            v1 = v1_p.tile([P, MT, NH, P], bf16)
            nc.gpsimd.memset(v1[:, :, 0::2, 64:128], 1.0)
            nc.gpsimd.memset(v1[:, :, 1::2, 0:64], 1.0)
            valT = [valT_p.tile([P, N], bf16, tag=f"valT{i}", name=f"valT{i}")
                    for i in range(2)]

            TAGS = ["A", "B", "V", "F"]

            # ---------- phase A: kt-major projection of q/k for pair 0 ----------
            with nc.named_scope("proj_qk02"):
                pt = {}
                for i, (ft, half) in enumerate([(0, 0), (0, 1), (2, 0), (2, 1)]):
                    pt[(ft, half)] = ps_p.tile(
                        [P, CH], f32, tag=TAGS[i], name=f"pqk_{ft}_{half}")
                for kt in range(KT):
                    # nb01 (first xT half) before nb23, matching the split DMA
                    for half in range(2):
                        for ft in (0, 2):
                            t = pt[(ft, half)]
                            for h2 in range(2):
                                nb = half * 2 + h2
                                nc.tensor.matmul(
                                    t[:, h2 * NB:(h2 + 1) * NB],
                                    lhsT=wqk_sb[:, kt, ft * P:(ft + 1) * P],
                                    rhs=xT_sb[:, kt, nb * NB:(nb + 1) * NB],
                                    start=(kt == 0), stop=(kt == KT - 1),
                                )
                for (ft, half), t in pt.items():
                    # ACT is idle during phase A: bias+copy in one activation
                    nc.scalar.activation(
                        out=qkT[:, ft, half * CH:(half + 1) * CH], in_=t,
                        func=AF.Identity, bias=bqk_sb[:, ft:ft + 1])

            # ---------- v projection: one m-tile (phase A seeds, rest are
            # pre-PV fillers inside window 0) ----------
            def pv_unit(mt, tag="F"):
                with nc.named_scope(f"proj_v_{mt}"):
                    t = ps_p.tile([P, NH * DV], f32, tag=tag)
                    for kt in range(KT):
                        nc.tensor.matmul(
                            t,
                            lhsT=xT_sb[:, kt, mt * P:(mt + 1) * P],
                            rhs=wv_sb[:, kt, :],
                            start=(kt == 0), stop=False,
                        )
                    nc.tensor.matmul(t, lhsT=ones1, rhs=bv_sb,
                                     start=False, stop=True)
                    psj = t.rearrange("p (j d) -> p j d", d=DV)
                    nc.vector.tensor_copy(
                        out=v1[:, mt, 0::2, 0:DV], in_=psj[:, 0::2, :])
                    nc.vector.tensor_copy(
                        out=v1[:, mt, 1::2, 64:64 + DV], in_=psj[:, 1::2, :])

            with nc.named_scope("proj_v_seed"):
                for mt in range(4):
                    pv_unit(mt, tag=TAGS[mt % 4])

            # ---------- filler units (run inside attention windows) ----------
            def qk_unit(ft, nb2, tag="F"):
                """Project one 256-wide n-block of q or k for head pair 1.

                Narrow fully-closed accumulation per filler slot: the PE
                chunk injected per slot (~0.85us) stays near the
                per-iteration ACT slack without open accumulation groups."""
                with nc.named_scope(f"fqk_{ft}_{nb2}"):
                    HB = NB // 2
                    t = ps_p.tile([P, HB], f32, tag=tag)
                    for kt in range(KT):
                        nc.tensor.matmul(
                            t,
                            lhsT=wqk_sb[:, kt, ft * P:(ft + 1) * P],
                            rhs=xT_sb[:, kt, nb2 * HB:(nb2 + 1) * HB],
                            start=(kt == 0), stop=(kt == KT - 1),
                        )
                    nc.vector.tensor_scalar_add(
                        out=qkT[:, ft, nb2 * HB:(nb2 + 1) * HB],
                        in0=t, scalar1=bqk_sb[:, ft:ft + 1])

            _op_i = [0]

            def op_unit(p, nt, tag="F", act_copy=None):
                """Output-project one 128-row n-block of head pair p."""
                with nc.named_scope(f"fop_{p}_{nt}"):
                    t = ps_p.tile([P, CH], f32, tag=tag)
                    for ds in range(2):
                        nc.tensor.matmul(
                            t[:, ds * NB:(ds + 1) * NB],
                            lhsT=valT[p][:, nt * P:(nt + 1) * P],
                            rhs=wout_sb[:, p, ds * NB:(ds + 1) * NB],
                            start=True, stop=True,
                        )
                    ob = stage_p.tile([P, CH], bf16)
                    # ACT copies only when ACT is idle (tail): mid-window they
                    # queue behind every remaining exp and stall the stores
                    if act_copy is None:
                        act_copy = False
                    if act_copy:
                        nc.scalar.copy(out=ob, in_=t)
                    else:
                        nc.vector.tensor_copy(out=ob, in_=t)
                    _op_i[0] += 1
                    nc.sync.dma_start(out=outp[p][nt * P:(nt + 1) * P, :], in_=ob)

            # ---------- attention window: one (pair, chunk, head) ----------
            # Windows alternate their PV accumulator between PSUM tags V and F
            # (double-buffered); fillers use the off-duty tag. Each window's
            # normalization is emitted inside the NEXT window (as a pre-PV
            # filler at iteration 2) so the window boundary never injects
            # DVE latency into the exp->mul->PV chain.
            _widx = [0]

            def attn(p, c, oe, fillers=(), pre=None, last=False, off=0, W=CH):
                j = 2 * p + oe
                base = oe * 64
                vtag = "VF"[_widx[0] % 2]
                ftag = "VF"[1 - _widx[0] % 2]
                _widx[0] += 1
                fill = {}
                if fillers:
                    # fillers reuse the previous window's psv PSUM slot: they
                    # must not be placed before the deferred norm (emitted at
                    # iteration 2) has registered its reads of that slot
                    n = len(fillers)
                    for i, f in enumerate(fillers):
                        mt_i = 3 + round(i * 11.0 / (n - 1)) if n > 1 else 8
                        fill.setdefault(min(14, mt_i), []).append(
                            lambda f=f: f(ftag))
                pre = pre or {}
                col = c * CH + off
                with nc.named_scope(f"attn_{p}_{c}_{oe}_{off}"):
                    psv = ps_p.tile([P, W], f32, tag=vtag,
                                    name=f"psv_{p}_{c}_{oe}_{off}")
                    for mt in range(MT):
                        psl = ps_p.tile([P, W], f32, tag="AB"[mt % 2])
                        for h2 in range(W // NB):
                            nc.tensor.matmul(
                                psl[:, h2 * NB:(h2 + 1) * NB],
                                lhsT=qkT[base:base + 64, 2 + p,
                                         mt * P:(mt + 1) * P],
                                rhs=qkT[base:base + 64, p,
                                        col + h2 * NB:col + (h2 + 1) * NB],
                                start=True, stop=True,
                            )
                        wt = wT_p.tile([P, W], bf16)
                        nc.scalar.activation(out=wt, in_=psl, func=AF.Exp)
                        nc.vector.tensor_mul(out=wt, in0=wt,
                                             in1=em_sb[:, c, mt, off:off + W])
                        for f in pre.get(mt, ()):
                            f(ftag)
                        for h2 in range(W // NB):
                            nc.tensor.matmul(
                                psv[:, h2 * NB:(h2 + 1) * NB],
                                lhsT=v1[:, mt, j, :],
                                rhs=wt[:, h2 * NB:(h2 + 1) * NB],
                                start=(mt == 0), stop=(mt == MT - 1),
                            )
                        for f in fill.get(mt, ()):
                            f()

                def finish(_tag=None):
                    # normalization: recip the replicated Z rows off psv,
                    # DMA-move across partitions (DVE is lane-locked), copy
                    # the value half out of PSUM, multiply on Pool.
                    with nc.named_scope(f"norm_{p}_{c}_{oe}_{off}"):
                        vb, zb = (0, 64) if oe == 0 else (64, 0)
                        cs = slice(col, col + W)
                        zr = z_p.tile([P, W], f32, tag="zr")
                        nc.vector.reciprocal(
                            out=zr[zb:zb + 64, :], in_=psv[zb:zb + 64, :])
                        if last:
                            # final window: move the recip block across
                            # partitions with an identity matmul (PE and PSUM
                            # are free here — ~0.2us vs ~3.3us for the DMA
                            # move) and multiply straight out of PSUM.
                            zmp = ps_p.tile([P, W], f32, tag=ftag, name="zmp")
                            nc.tensor.matmul(
                                zmp[vb:vb + 64, :],
                                lhsT=ident_sb[zb:zb + 64, :],
                                rhs=zr[zb:zb + 64, :],
                                start=True, stop=True,
                            )
                            # only one PSUM operand allowed per DVE op: the
                            # value half goes through SBUF
                            vc = z_p.tile([P, W], f32, tag="pc")
                            nc.vector.tensor_copy(
                                out=vc[vb:vb + 64, :], in_=psv[vb:vb + 64, :])
                            h = W // 2
                            nc.vector.tensor_mul(
                                out=valT[p][vb:vb + 64, cs][:, 0:h],
                                in0=vc[vb:vb + 64, 0:h],
                                in1=zmp[vb:vb + 64, 0:h])
                            nc.vector.tensor_mul(
                                out=valT[p][vb:vb + 64, cs][:, h:W],
                                in0=vc[vb:vb + 64, h:W],
                                in1=zmp[vb:vb + 64, h:W])
                        else:
                            zm = z_p.tile([P, W], f32, tag="zm")
                            # issue via SWDGE (gpsimd): never queues behind
                            # the output stores on the sync queue
                            nc.gpsimd.dma_start(
                                out=zm[vb:vb + 64, :], in_=zr[zb:zb + 64, :])
                            vc = z_p.tile([P, W], f32, tag="pc")
                            nc.vector.tensor_copy(
                                out=vc[vb:vb + 64, :], in_=psv[vb:vb + 64, :])
                            nc.gpsimd.tensor_mul(
                                out=valT[p][vb:vb + 64, cs],
                                in0=vc[vb:vb + 64, :], in1=zm[vb:vb + 64, :])

                if last:
                    finish()
                    return None
                return finish

            # ---------- window schedule with interleaved fillers ----------
            # window 0 carries the rest of the v projection as pre-PV fillers
            # (v1[mt] must exist before its own PV consumes it; 4-ahead lead)
            def qk_f(ft, nb):
                return lambda tag: qk_unit(ft, nb, tag)

            def op_f(p, nt):
                return lambda tag: op_unit(p, nt, tag)

            n0 = attn(0, 0, 0,
                      pre={mt: [lambda tag, mt=mt: pv_unit(mt + 4, tag)]
                           for mt in range(MT - 4)})
            n1 = attn(0, 0, 1, [qk_f(1, nb2) for nb2 in range(8)],
                      pre={2: [n0]})
            n2 = attn(0, 1, 0, [qk_f(3, nb2) for nb2 in range(4)]
                      + [op_f(0, 0), op_f(0, 1)],
                      pre={2: [n1]})
            n3 = attn(0, 1, 1, [qk_f(3, nb2) for nb2 in range(4, 8)]
                      + [op_f(0, 2), op_f(0, 3)],
                      pre={2: [n2]})
            n4 = attn(1, 0, 0, [op_f(0, nt) for nt in (4, 5, 6, 7, 8, 9)],
                      pre={2: [n3]})
            n5 = attn(1, 0, 1, [op_f(0, nt) for nt in (10, 11, 12, 13, 14, 15)],
                      pre={2: [n4]})
            n6 = attn(1, 1, 0, [op_f(1, nt) for nt in range(6)],
                      pre={2: [n5]})
            # final window splits into two 512-wide halves: half A's norm and
            # output projection overlap half B's compute, shrinking the tail
            n7a = attn(1, 1, 1, [op_f(1, 6), op_f(1, 7)],
                       pre={2: [n6]}, off=0, W=NB)
            attn(1, 1, 1, [op_f(1, nt) for nt in (8, 9, 10, 11)],
                 pre={2: [n7a]}, last=True, off=NB, W=NB)
            # tail: final 512 columns of pair 1, rotating through freed PSUM
            # tags; copies alternate DVE/ACT (both idle by now)
            with nc.named_scope("op_tail"):
                for i, nt in enumerate(range(12, 16)):
                    op_unit(1, nt, tag=["A", "B", "V", "F"][i % 4],
                            act_copy=(i % 2 == 1))

    nc.compile()
    return nc


def get_nc():
    if "nc" not in _BUILT:
        _BUILT["nc"] = _build_nc()
    return _BUILT["nc"]


def _shard_inputs(stream, mask, w_qkv, b_qkv, w_out):
    """Build the 8 per-core input maps (host-side layout transforms)."""
    stream = np.asarray(stream, np.float32)
    mask = np.asarray(mask, np.float32)
    w_qkv = np.asarray(w_qkv, np.float32)
    b_qkv = np.asarray(b_qkv, np.float32)
    w_out = np.asarray(w_out, np.float32)

    emT = np.exp(mask[0].T).astype(BF16)  # (N, N) exp of transposed mask
    xT = [np.ascontiguousarray(stream[:, b, :].T).astype(BF16) for b in range(B)]

    in_maps = []
    for d in range(N_CORES):
        b = d // 4
        heads = [(d % 4) * 4 + j for j in range(NH)]
        qc = [w_qkv[:, h * DHEAD:h * DHEAD + DKQ] for h in heads]
        kc = [w_qkv[:, h * DHEAD + DKQ:h * DHEAD + 2 * DKQ] for h in heads]
        vc = [w_qkv[:, h * DHEAD + 2 * DKQ:(h + 1) * DHEAD] for h in heads]
        wqk = np.ascontiguousarray(np.concatenate(
            [qc[0], qc[1], qc[2], qc[3], kc[0], kc[1], kc[2], kc[3]],
            axis=1)).astype(BF16)
        wv = np.ascontiguousarray(np.concatenate(vc, axis=1)).astype(BF16)
        bq = [b_qkv[h * DHEAD:h * DHEAD + DKQ] for h in heads]
        bk = [b_qkv[h * DHEAD + DKQ:h * DHEAD + 2 * DKQ] for h in heads]
        bvv = [b_qkv[h * DHEAD + 2 * DKQ:(h + 1) * DHEAD] for h in heads]
        bqk_arr = np.stack(
            [np.concatenate([bq[0], bq[1]]), np.concatenate([bq[2], bq[3]]),
             np.concatenate([bk[0], bk[1]]), np.concatenate([bk[2], bk[3]])],
            axis=1).astype(np.float32)
        bv_arr = np.ascontiguousarray(np.concatenate(bvv)[None, :]).astype(BF16)
        woutd = np.ascontiguousarray(np.concatenate(
            [w_out[h * DV:(h + 1) * DV, :] for h in heads], axis=0)).astype(BF16)
        in_maps.append({
            "xT": xT[b], "wqk": wqk, "wv": wv, "bqk": bqk_arr, "bv": bv_arr,
            "ones": np.ones((1, P), BF16), "emT": emT, "wout": woutd,
            "ident": np.vstack([np.eye(64), np.eye(64)]).astype(np.float32),
        })
    return in_maps


def kernel(stream, mask, w_qkv, b_qkv, w_out, b_out):
    nc = get_nc()
    in_maps = _shard_inputs(stream, mask, w_qkv, b_qkv, w_out)
    res = run_bass_kernel_spmd(nc, in_maps, core_ids=list(range(N_CORES)))
    b_out = np.asarray(b_out, np.float32)
    out = np.empty((N, B, DSTR), np.float32)
    for b in range(B):
        acc = res.results[4 * b]["out0"].astype(np.float32)
        acc += res.results[4 * b]["out1"].astype(np.float32)
        for i in range(1, 4):
            acc += res.results[4 * b + i]["out0"].astype(np.float32)
            acc += res.results[4 * b + i]["out1"].astype(np.float32)
        out[:, b, :] = acc + b_out
    return out
